# revision 3
# baseline (speedup 1.0000x reference)
"""Character-LSTM Trainium2 kernel V2 (8 NeuronCores, SPMD data-parallel).

Strategy
--------
Words sorted by descending length are dealt round-robin across 8 cores so the
per-step active-column count A[t] is core-uniform; words live as columns of
transposed state tiles. Per step, gates g[4H x cols] are computed on the PE
entirely in Double-FP8 (2 MACs/cell/cycle): the x-part uses a host-built fp8
one-hot of the char against a hi/lo pair of fp8 embproj tables
(emb@W_ih.T + bias, value + e4m3 residual -> ~f16 accuracy at fp8 speed), and
the h-part uses fp8 W_hh x fp8 h. Each DoubleRow matmul contracts K=256 in one
pass. Activations run on ACT as one instruction per gate (both 128-row chunks
via a 2D access pattern) with the x16 table scaling folded into the ACT scale;
bias is folded into the embproj tables. The cell update runs on DVE in f16
(2x mode); h is stored fp8 for the next step's matmuls, and in fp32 only for
columns finishing at this step (the output). PE sweeps that do not depend on
h (x-part) are emitted before the h sweeps to keep the recurrence critical
path short at step boundaries, and finished columns stream to HBM per step.
"""

import sys

if "/opt/trn_rl_repo" not in sys.path:
    sys.path.insert(0, "/opt/trn_rl_repo")

import contextlib

import numpy as np
import ml_dtypes

import concourse.bass as bass
import concourse.tile as tile
from concourse import bacc, mybir
from concourse.bass_utils import run_bass_kernel_spmd

E4 = ml_dtypes.float8_e4m3fn
NCORES = 8
B, S, W, E, H, V = 64, 256, 24, 128, 256, 256
GATE_FUNCS = ["Sigmoid", "Sigmoid", "Tanh", "Sigmoid"]  # i, f, g, o
GORDER = (0, 2, 1, 3)  # emission order: i, g, f, o
SC = 16.0  # fp8 table scale; ACT applies 1/SC
CH = 512  # column chunk (PSUM: 4 gate tags x [128, 2*CH] fp32 = 8 banks)
MMW = 256  # DoubleRow matmul window (moving free = 2*MMW = 512 = max)

_PROGRAM_CACHE: dict = {}


def _plan(lens: np.ndarray, round_to: int = 1):
    """Column counts per step, uniform across cores."""
    wL = np.bincount(lens, minlength=W + 1)
    colsL = np.zeros(W + 1, np.int64)
    cum = 0
    for L in range(W, 0, -1):
        need = -(-int(wL[L]) // NCORES)
        newcum = -(-(cum + need) // round_to) * round_to
        colsL[L] = newcum - cum
        cum = newcum
    C = max(cum, 16)
    A = [int(colsL[t + 1 :].sum()) for t in range(W)]
    return colsL, C, A


def _assign(lens, chars, colsL, C):
    """Deal words into (core, column) slots, longest first."""
    order = np.argsort(-lens, kind="stable")
    wL = np.bincount(lens, minlength=W + 1)
    colmap = np.full((NCORES, C), -1, np.int64)
    col_chars = np.zeros((NCORES, C, W), np.int64)
    pos = 0
    s = 0
    for L in range(W, 0, -1):
        cnt = int(wL[L])
        if cnt:
            ids = order[pos : pos + cnt]
            pos += cnt
            k = np.arange(cnt) % NCORES
            j = s + np.arange(cnt) // NCORES
            colmap[k, j] = ids
            col_chars[k, j] = chars[ids]
        s += int(colsL[L])
    return colmap, col_chars


def _pl(t, stride, lo, w):
    """AP over tile t: [128 part, 2 planes (stride), w cols] at col offset lo."""
    return bass.AP(
        tensor=t.tensor, offset=t.offset + lo, ap=[t.ap[0], [stride, 2], [1, w]]
    )


def _build_program(C: int, A: list[int], reps: int = 1, ch: int = CH, mmw: int = MMW,
                   hsplit: int = 1, h8_pool: bool = False, merge: int = 0):
    key = (C, tuple(A), reps, ch, mmw, hsplit, h8_pool, merge)
    if key in _PROGRAM_CACHE:
        return _PROGRAM_CACHE[key]
    assert not merge or ch == 512

    dt = mybir.dt
    AF = mybir.ActivationFunctionType
    DR = mybir.MatmulPerfMode.DoubleRow
    nc = bacc.Bacc("TRN2", target_bir_lowering=False, debug=False, num_devices=NCORES)

    oh_d = nc.dram_tensor("oh", [W, 128, 2 * C], dt.float8e4, kind="ExternalInput")
    eph_d = nc.dram_tensor("eph", [128, 2 * 4 * H], dt.float8e4, kind="ExternalInput")
    epl_d = nc.dram_tensor("epl", [128, 2 * 4 * H], dt.float8e4, kind="ExternalInput")
    whh_d = nc.dram_tensor("whh", [128, 2 * 4 * H], dt.float8e4, kind="ExternalInput")
    out_d = nc.dram_tensor("out", [2, 128, C], dt.float32, kind="ExternalOutput")

    with tile.TileContext(nc) as tc:
        with (
            tc.tile_pool(name="const", bufs=1) as constp,
            tc.tile_pool(name="state", bufs=1) as statep,
            tc.tile_pool(name="ohp", bufs=3) as ohp,
            tc.tile_pool(name="gates", bufs=3) as gatesp,
            tc.tile_pool(name="work", bufs=3) as workp,
            tc.tile_pool(name="psum", bufs=1, space="PSUM") as psump,
        ):
            eph_sb = constp.tile([128, 2 * 4 * H], dt.float8e4, tag="eph")
            epl_sb = constp.tile([128, 2 * 4 * H], dt.float8e4, tag="epl")
            whh_sb = constp.tile([128, 2 * 4 * H], dt.float8e4, tag="whh")
            nc.sync.dma_start(out=eph_sb, in_=eph_d[:])
            nc.sync.dma_start(out=epl_sb, in_=epl_d[:])
            nc.sync.dma_start(out=whh_sb, in_=whh_d[:])

            h8 = statep.tile([128, 2 * C], dt.float8e4, tag="h8")
            cst = statep.tile([128, 2 * C], dt.float16, tag="cst")
            hf = statep.tile([128, 2 * C], dt.float32, tag="hf")
            nc.vector.memset(hf[:], 0.0)

            def stat_ap(tbl, m):
                # stationary [K=128, 2 planes, 128] for gate-dim chunk m
                return bass.AP(
                    tensor=tbl.tensor,
                    offset=tbl.offset + m * 128,
                    ap=[tbl.ap[0], [4 * H, 2], [1, 128]],
                )

            def emit_iteration():
                for t in range(W):
                    At = A[t]
                    if At == 0:
                        break
                    Atn = A[t + 1] if t + 1 < W else 0
                    first = t == 0

                    oh = ohp.tile([128, 2 * C], dt.float8e4, tag="oh")
                    src = oh_d[t]
                    nc.sync.dma_start(
                        out=_pl(oh, C, 0, At),
                        in_=bass.AP(
                            tensor=src.tensor,
                            offset=src.offset,
                            ap=[src.ap[0], [C, 2], [1, At]],
                        ),
                    )

                    nq = -(-At // ch)
                    for q in range(nq):
                        qlo = q * ch
                        bq = min(ch, At - qlo)
                        nw = -(-bq // mmw)
                        gact = GORDER if not first else (0, 2, 3)
                        # 4 PSUM tags at ch<=512, 2 phase-shared tags at ch=1024
                        four_tags = 2 * ch * 4 * len(GORDER) <= 16384
                        phases = (
                            [tuple(gact)]
                            if four_tags
                            else [tuple(g for g in (0, 2) if g in gact),
                                  tuple(g for g in (1, 3) if g in gact)]
                        )

                        pst = {}
                        gts = {}

                        def gref(g, lo, w):
                            gt, base = gts[g]
                            return bass.AP(
                                tensor=gt.tensor,
                                offset=gt.offset + base + lo,
                                ap=[gt.ap[0], [ch, 2], [1, w]],
                            )

                        def sweep(g, tbl, hi, sp):
                            pt, base = pst[g]
                            for msub in range(2):
                                sap = stat_ap(tbl, g * 2 + msub)
                                for wdx in range(nw):
                                    wlo = qlo + wdx * mmw
                                    bw = min(mmw, bq - wdx * mmw)
                                    mv = (
                                        _pl(h8, C, wlo, bw)
                                        if tbl is whh_sb
                                        else _pl(oh, C, wlo, bw)
                                    )
                                    off = base + msub * ch + wdx * mmw
                                    # start=True zeroes the full 2KB PSUM bank:
                                    # assert only on bank-aligned windows
                                    st = hi and off % 512 == 0
                                    nc.tensor.matmul(
                                        pt[:, off : off + bw],
                                        sap,
                                        mv,
                                        start=st,
                                        stop=sp,
                                        perf_mode=DR,
                                    )

                        if merge == 2:
                            TA = psump.tile([128, 2048], dt.float32, tag="TA", name="TA")
                            TB = psump.tile([128, 2048], dt.float32, tag="TB", name="TB")
                            goff = {0: 0, 1: 1024, 2: 0, 3: 1024}
                            pst = {g: (TA if g in (0, 1) else TB, goff[g]) for g in gact}
                            for g in gact:
                                sweep(g, eph_sb, True, False)
                                sweep(g, epl_sb, False, first)
                            if not first:
                                for g in gact:
                                    sweep(g, whh_sb, False, True)
                            gif = gatesp.tile([128, 2048], dt.float16, tag="gif")
                            ggo = gatesp.tile([128, 2048], dt.float16, tag="ggo")
                            gts = {0: (gif, 0), 1: (gif, 1024), 2: (ggo, 0), 3: (ggo, 1024)}

                            def bigap(tl, base, planes, w):
                                return bass.AP(
                                    tensor=tl.tensor,
                                    offset=tl.offset + base,
                                    ap=[tl.ap[0], [512, planes], [1, w]],
                                )

                            if not first:
                                nc.scalar.activation(
                                    bigap(gif, 0, 4, bq), bigap(TA, 0, 4, bq),
                                    AF.Sigmoid, scale=1.0 / SC,
                                )
                            else:
                                nc.scalar.activation(
                                    bigap(gif, 0, 2, bq), bigap(TA, 0, 2, bq),
                                    AF.Sigmoid, scale=1.0 / SC,
                                )
                            nc.scalar.activation(
                                bigap(ggo, 0, 2, bq), bigap(TB, 0, 2, bq),
                                AF.Tanh, scale=1.0 / SC,
                            )
                            nc.scalar.activation(
                                bigap(ggo, 1024, 2, bq), bigap(TB, 1024, 2, bq),
                                AF.Sigmoid, scale=1.0 / SC,
                            )
                            if not first:
                                ig = workp.tile([128, 2 * ch], dt.float16, tag="ig")
                                nc.vector.tensor_mul(
                                    _pl(ig, ch, 0, bq), gref(0, 0, bq), gref(2, 0, bq)
                                )
                        elif merge:
                            # one PSUM tile; i, f, o contiguous so one Sigmoid
                            # instruction covers them; g separate (Tanh)
                            big = psump.tile(
                                [128, 4096], dt.float32, tag="PS", name="PS"
                            )
                            goff = {0: 0, 1: 1024, 3: 2048, 2: 3072}
                            pst = {g: (big, goff[g]) for g in gact}
                            for g in gact:
                                sweep(g, eph_sb, True, False)
                                sweep(g, epl_sb, False, first)
                            if not first:
                                for g in gact:
                                    sweep(g, whh_sb, False, True)
                            gifo = gatesp.tile([128, 3072], dt.float16, tag="gifo")
                            ggt = gatesp.tile([128, 1024], dt.float16, tag="ggt")
                            gts = {0: (gifo, 0), 1: (gifo, 1024), 3: (gifo, 2048),
                                   2: (ggt, 0)}

                            def bigap(tl, base, planes, w):
                                return bass.AP(
                                    tensor=tl.tensor,
                                    offset=tl.offset + base,
                                    ap=[tl.ap[0], [512, planes], [1, w]],
                                )

                            if not first:
                                nc.scalar.activation(
                                    bigap(gifo, 0, 6, bq), bigap(big, 0, 6, bq),
                                    AF.Sigmoid, scale=1.0 / SC,
                                )
                            else:
                                nc.scalar.activation(
                                    bigap(gifo, 0, 2, bq), bigap(big, 0, 2, bq),
                                    AF.Sigmoid, scale=1.0 / SC,
                                )
                                nc.scalar.activation(
                                    bigap(gifo, 2048, 2, bq), bigap(big, 2048, 2, bq),
                                    AF.Sigmoid, scale=1.0 / SC,
                                )
                            nc.scalar.activation(
                                bigap(ggt, 0, 2, bq), bigap(big, 3072, 2, bq),
                                AF.Tanh, scale=1.0 / SC,
                            )
                            if not first:
                                ig = workp.tile([128, 2 * ch], dt.float16, tag="ig")
                                nc.vector.tensor_mul(
                                    _pl(ig, ch, 0, bq), gref(0, 0, bq), gref(2, 0, bq)
                                )
                        else:
                            for phase in phases:
                                for g in phase:
                                    tagn = f"T{g}" if four_tags else f"T{0 if g in (0, 1) else 1}"
                                    pst[g] = (
                                        psump.tile(
                                            [128, 2 * ch], dt.float32,
                                            tag=tagn, name=tagn,
                                        ),
                                        0,
                                    )
                                # x-part sweeps first (independent of h)
                                for g in phase:
                                    sweep(g, eph_sb, True, False)
                                    sweep(g, epl_sb, False, first)
                                if not first:
                                    for g in phase:
                                        sweep(g, whh_sb, False, True)
                                for g in phase:
                                    gt = gatesp.tile(
                                        [128, 2 * ch], dt.float16,
                                        tag=f"g{g}", name=f"g{g}",
                                    )
                                    nc.scalar.activation(
                                        _pl(gt, ch, 0, bq),
                                        _pl(pst[g][0], ch, 0, bq),
                                        getattr(AF, GATE_FUNCS[g]),
                                        scale=1.0 / SC,
                                    )
                                    gts[g] = (gt, 0)
                                if 0 in phase and 2 in phase and not first:
                                    ig = workp.tile(
                                        [128, 2 * ch], dt.float16, tag="ig"
                                    )
                                    nc.vector.tensor_mul(
                                        _pl(ig, ch, 0, bq),
                                        gref(0, 0, bq),
                                        gref(2, 0, bq),
                                    )

                        cwin = _pl(cst, C, qlo, bq)
                        if first:
                            nc.vector.tensor_mul(cwin, gref(0, 0, bq), gref(2, 0, bq))

                        th = workp.tile([128, 2 * ch], dt.float16, tag="th")
                        hb = max(0, min(Atn - qlo, bq))
                        hs_eff = hsplit if bq > 256 else 1
                        hw_w = -(-bq // hs_eff)
                        for hlo in range(0, bq, hw_w):
                            bh = min(hw_w, bq - hlo)
                            if not first:
                                fc = workp.tile(
                                    [128, 2 * ch], dt.float16, tag="fc"
                                )
                                nc.vector.tensor_mul(
                                    _pl(fc, ch, hlo, bh),
                                    gref(1, hlo, bh),
                                    _pl(cst, C, qlo + hlo, bh),
                                )
                                nc.vector.tensor_add(
                                    _pl(cst, C, qlo + hlo, bh),
                                    _pl(ig, ch, hlo, bh),
                                    _pl(fc, ch, hlo, bh),
                                )
                            nc.scalar.activation(
                                _pl(th, ch, hlo, bh),
                                _pl(cst, C, qlo + hlo, bh),
                                AF.Tanh,
                            )
                            # h for next step (fp8) / finished columns (fp32)
                            ah = max(0, min(hb - hlo, bh))
                            if ah > 0:
                                h8_eng = nc.gpsimd if h8_pool else nc.vector
                                h8_eng.tensor_mul(
                                    _pl(h8, C, qlo + hlo, ah),
                                    gref(3, hlo, ah),
                                    _pl(th, ch, hlo, ah),
                                )
                            if bh - ah > 0:
                                nc.vector.tensor_mul(
                                    _pl(hf, C, qlo + hlo + ah, bh - ah),
                                    gref(3, hlo + ah, bh - ah),
                                    _pl(th, ch, hlo + ah, bh - ah),
                                )

                    # stream out the columns that finished at this step
                    fin_lo, fin_hi = Atn, At
                    if fin_hi > fin_lo:
                        for p in range(2):
                            base = out_d[p]
                            nc.sync.dma_start(
                                out=bass.AP(
                                    tensor=base.tensor,
                                    offset=base.offset + fin_lo,
                                    ap=[base.ap[0], [1, fin_hi - fin_lo]],
                                ),
                                in_=bass.AP(
                                    tensor=hf.tensor,
                                    offset=hf.offset + p * C + fin_lo,
                                    ap=[hf.ap[0], [1, fin_hi - fin_lo]],
                                ),
                            )

            if reps == 1:
                emit_iteration()
            else:
                with tc.For_i(0, reps, 1):
                    emit_iteration()

    nc.compile()
    _PROGRAM_CACHE[key] = nc
    return nc


def _prepare(char_input, embedding, W_ih, W_hh, b_ih, b_hh, round_to=2):
    ci = np.asarray(char_input)
    chars = ci.reshape(-1, W).astype(np.int64)
    lens = (chars != 0).sum(-1)

    colsL, C, A = _plan(lens, round_to)
    colmap, col_chars = _assign(lens, chars, colsL, C)

    ep = (
        np.asarray(embedding, np.float64) @ np.asarray(W_ih, np.float64).T
        + np.asarray(b_ih, np.float64)
        + np.asarray(b_hh, np.float64)
    ).astype(np.float32)  # [V, 4H], bias folded
    t1 = np.clip(ep * SC, -240, 240).astype(E4)
    t2 = np.clip(ep * SC - t1.astype(np.float32), -240, 240).astype(E4)

    def dr_layout(tbl):  # [V, 4H] -> [128, 2*4H], k=(p, plane): v = plane*128+p
        return np.ascontiguousarray(
            tbl.reshape(2, 128, 4 * H).transpose(1, 0, 2).reshape(128, 2 * 4 * H)
        )

    whh_q = np.clip(np.asarray(W_hh, np.float32).T * SC, -240, 240).astype(E4)

    common = {
        "eph": dr_layout(t1),
        "epl": dr_layout(t2),
        "whh": dr_layout(whh_q),
    }
    in_maps = []
    tgrid = np.broadcast_to(np.arange(W)[:, None], (W, C))
    cgrid = np.broadcast_to(np.arange(C)[None, :], (W, C))
    for k in range(NCORES):
        v = col_chars[k].T  # [W, C]
        oh = np.zeros((W, 128, 2 * C), np.uint8)
        oh[tgrid, v % 128, (v // 128) * C + cgrid] = 0x38  # 1.0 in e4m3
        in_maps.append({"oh": oh.view(E4), **common})
    return colmap, in_maps, C, A


def _gather_output(results, colmap):
    out_flat = np.zeros((B * S, H), np.float32)
    for k in range(NCORES):
        o = results[k]["out"].astype(np.float32)  # [2, 128, C]
        h_core = o.reshape(H, o.shape[-1])
        mask = colmap[k] >= 0
        out_flat[colmap[k][mask]] = h_core[:, mask].T
    return out_flat.reshape(B, S, H)


def kernel(char_input, embedding, W_ih, W_hh, b_ih, b_hh):
    colmap, in_maps, C, A = _prepare(
        char_input, embedding, W_ih, W_hh, b_ih, b_hh, round_to=2
    )
    nc = _build_program(C, A)
    res = run_bass_kernel_spmd(nc, in_maps, core_ids=list(range(NCORES)))
    return _gather_output(res.results, colmap)
